# revision 27
# baseline (speedup 1.0000x reference)
"""DepthCueExtractor TRN2 kernel (bf16 I/O, collective-free).

out[b,u,y,x,n] = sum_v(lfi[b,u,y,x,v]) * wf[b,y,n]
  wf[b,y,n]     = colsum[b,y,n] * s_mask[b,n] / (V * max_w colsum[b,w,n])
  s_mask[b,n]   = sum_{h,w} f_maps[b,h,w,n]
  colsum[b,w,n] = sum_h f_maps[b,h,w,n]   (h==w==256 so w doubles as y)

Sharding: 8 cores = (batch b) x (H-half), data-parallel on the output with no
cross-core communication: each core redundantly reads BOTH 128-column W-halves
of f_maps[b] (+4.2MB that hides under the 37.8MB store stream, cheaper than
the cost model's 15us flat collective overhead), computing the global
per-(b,n) sum/max stats locally.

Everything runs in bf16: inputs are cast on the host (stats still accumulate
in f32 on the PE/PSUM path so only the lfi samples and final product round),
and the output is stored bf16, halving the dominant HBM write stream.
Per-core traffic 51.4MB -> ~143us at the 360GB/s DMA roofline.

colsum is built by ones-stationary PE matmuls accumulating both h-halves into
[1, 2048] PSUM rows (no DVE work; 6 row slots at partition bases 0/32/64 of
two [65, 2048] tiles so rows never wait on each other), evacuated row-by-row
on the otherwise-idle Act engine and scatter-DMA'd into [w-partition, n]
layout via Act's HWDGE queue (so the tiny scatters never head-block the SP
load queue).

The output is laid out [U, Y, N, X] on device (host transposes during the
gather): with x innermost, every operand of the big product - mlf[y,x]
broadcast over an OUTER n dim, and wf[y,n] pre-replicated over x into
WREP[y,n,x] by log-doubling copies - is innermost-packed 2-byte, which
qualifies for the DVE 2x perf mode (0.52ns/elem). One output tile per
u-group goes to the idle Pool engine and the remaining V-sum reduces are
wait_until-pinned near their consumers, keeping the DVE multiply stream
paced to the store stream.
"""

import numpy as np
import ml_dtypes

import concourse.bass as bass
import concourse.bacc as bacc
import concourse.mybir as mybir
import concourse.tile as tile
from concourse.bass_utils import run_bass_kernel_spmd

F32 = mybir.dt.float32
BF16 = mybir.dt.bfloat16
NP_BF16 = ml_dtypes.bfloat16

B, U, H, W, V, N = 4, 9, 256, 256, 9, 64
HY = H // 2
NC = 16  # n-chunk width of one output tile [128, NC, W]
POOL_TILES = None  # resolved in build_kernel_body from NC
EVAC_SPLIT = False   # odd colsum rows evacuated on DVE instead of Act
EVAC_POOL = False    # odd colsum rows evacuated on the idle Pool engine
RED_SPLIT = False    # split V-sum reduces into 4 x-chunks to fit store slack
FM_BUFS = 2
WREP_EAGER = False   # build WREP chunks 1-3 right after first emit
PIN_BASE = 0.034     # ms: wait_until pin for reduce_u(u+1)
PIN_STEP = 0.0115
OUT_BUFS = 7
LT_BUFS = 9
ST_BUFS = 3
WREP_PIN = 0.0      # ms: if >0, pin wrep chunk c>=1 builds at WREP_PIN + 0.0025*c


def build_kernel_body(nc, tc, lfi_s, fm, out_s):
    import concourse.bass_isa as bass_isa

    pool_tiles = POOL_TILES
    if pool_tiles is None:
        if NC == 16:
            pool_tiles = {(u, 1) for u in range(1, 9)}
        else:
            pool_tiles = {(u, 1) for u in (1, 3, 5, 7)}
    with (
        tc.tile_pool(name="const", bufs=1) as const_pool,
        tc.tile_pool(name="fmp", bufs=1) as fm_pool,
        tc.tile_pool(name="psum", bufs=1, space="PSUM") as psum_pool,
        tc.tile_pool(name="evac", bufs=1) as evac_pool,
        tc.tile_pool(name="stats", bufs=1) as stats_pool,
        tc.tile_pool(name="lfip", bufs=3) as lfi_pool,
        tc.tile_pool(name="mlfp", bufs=1) as mlf_pool,
        tc.tile_pool(name="wrepp", bufs=1) as wrep_pool,
        tc.tile_pool(name="outp", bufs=2) as out_pool,
    ):
        ones = const_pool.tile([128, 1], BF16)
        nc.vector.memset(ones[:], 1.0)

        cs0 = stats_pool.tile([128, N], F32)  # colsum, my w-half (= my y's)
        cs1 = stats_pool.tile([128, N], F32)  # colsum, partner w-half

        # 6 PSUM row slots at partition bases 0/32/64 of two [65, 2048] tiles:
        # every colsum row's matmuls can run as soon as its fm half lands,
        # instead of chaining behind a prior row's evacuation.
        ps_a = psum_pool.tile([65, 32 * N], F32)
        ps_b = psum_pool.tile([65, 32 * N], F32)

        def row_slot(row):
            s = row % 6
            return (ps_a if s < 3 else ps_b), 32 * (s % 3)

        # ---- Phase A: colsum[w, n] = sum_h fm[h, w, n] for all 256 w.
        # Per 64-w chunk: load both h-halves, accumulate them into [1, 2048]
        # PSUM rows with ones-stationary matmuls, copy each row to SBUF on
        # Act, scatter-DMA it to 32 partitions of cs0/cs1.
        def phase_a_chunk(q):
            f0 = fm_pool.tile([128, 64, N], BF16, name=f"f0_{q}", tag="f0", bufs=FM_BUFS)
            f1 = fm_pool.tile([128, 64, N], BF16, name=f"f1_{q}", tag="f1", bufs=FM_BUFS)
            for h in range(2):
                sl = slice(q * 64 + h * 32, q * 64 + (h + 1) * 32)
                tl = slice(h * 32, (h + 1) * 32)
                nc.sync.dma_start(out=f0[:, tl, :], in_=fm[0:128, sl, :])
                nc.sync.dma_start(out=f1[:, tl, :], in_=fm[128:256, sl, :])
            for r2 in range(2):
                row = 2 * q + r2
                pt, pb = row_slot(row)
                rt = pt[pb : pb + 1, :]
                for sg in range(4):
                    osl = slice(sg * 512, (sg + 1) * 512)
                    rsl = slice(r2 * 32 + sg * 8, r2 * 32 + (sg + 1) * 8)
                    nc.tensor.matmul(
                        out=rt[0:1, osl], lhsT=ones[:, 0:1],
                        rhs=f0[:, rsl, :], start=True, stop=False,
                    )
                    nc.tensor.matmul(
                        out=rt[0:1, osl], lhsT=ones[:, 0:1],
                        rhs=f1[:, rsl, :], start=False, stop=True,
                    )
                st = evac_pool.tile(
                    [1, 32 * N], F32, name=f"st{row}", tag="st", bufs=ST_BUFS
                )
                # alternate evac rows between Act and (otherwise idle) DVE so
                # the eight 1.9us copies don't serialize on one engine
                if row % 2 == 1 and EVAC_POOL:
                    nc.gpsimd.tensor_copy(out=st[:], in_=rt[0:1, :])
                elif row % 2 == 1 and EVAC_SPLIT:
                    nc.vector.tensor_copy(out=st[:], in_=rt[0:1, :])
                else:
                    nc.scalar.copy(out=st[:], in_=rt[0:1, :])
                dst = cs0 if row < 4 else cs1
                pbase = (row % 4) * 32
                # Act's HWDGE queue: keeps these off the SP queue so they
                # never head-block the input load stream.
                nc.scalar.dma_start(out=dst[pbase : pbase + 32, :], in_=st[0:1, :])

        # All fm chunks load FIRST: the colsum/stats path (which gates the
        # first store through wf->WREP) completes while the lfi loads are
        # still streaming, so the store stream starts before inputs finish.
        for q in range(4):
            phase_a_chunk(q)

        lts = []

        def load_lt(u):
            # bufs=9: every lfi tile stays live; reduces are pinned late, so
            # recycling would stall the load stream.
            lt = lfi_pool.tile([128, W, V], BF16, name=f"lt{u}", tag="lt", bufs=LT_BUFS)
            nc.sync.dma_start(out=lt[:], in_=lfi_s[u])
            lts.append(lt)

        for u in range(U):
            load_lt(u)

        mlf32 = [
            mlf_pool.tile([128, W], F32, name=f"m32_{u}", tag="m32", bufs=2)
            for u in range(U)
        ]
        mlfb = [
            mlf_pool.tile([128, W], BF16, name=f"mb{u}", tag=f"mb{u}")
            for u in range(U)
        ]

        def reduce_u(u):
            if RED_SPLIT:
                for xc in range(4):
                    xs = slice(xc * 64, (xc + 1) * 64)
                    nc.vector.reduce_sum(
                        out=mlf32[u][:, xs], in_=lts[u][:, xs, :],
                        axis=mybir.AxisListType.X,
                    )
                    nc.vector.tensor_copy(
                        out=mlfb[u][:, xs], in_=mlf32[u][:, xs]
                    )
            else:
                nc.vector.reduce_sum(
                    out=mlf32[u][:], in_=lts[u][:], axis=mybir.AxisListType.X
                )
                nc.vector.tensor_copy(out=mlfb[u][:], in_=mlf32[u][:])

        reduce_u(0)  # on the first-store critical path; the rest are pinned

        hp = tc.high_priority

        # ---- stats: global sum/max over w, then wf = cs_my * s / (V * max)
        with hp():
            t_sum = stats_pool.tile([128, N], F32)
            nc.vector.tensor_add(out=t_sum[:], in0=cs0[:], in1=cs1[:])
            t_max = stats_pool.tile([128, N], F32)
            nc.vector.tensor_max(out=t_max[:], in0=cs0[:], in1=cs1[:])
            s_all = stats_pool.tile([128, N], F32)
            nc.gpsimd.partition_all_reduce(
                s_all[:], t_sum[:], 128, bass_isa.ReduceOp.add
            )
            m_all = stats_pool.tile([128, N], F32)
            nc.gpsimd.partition_all_reduce(
                m_all[:], t_max[:], 128, bass_isa.ReduceOp.max
            )
            m9 = stats_pool.tile([128, N], F32)
            nc.vector.tensor_scalar_mul(m9[:], m_all[:], float(V))
            rec = stats_pool.tile([128, N], F32)
            nc.vector.reciprocal(out=rec[:], in_=m9[:])
            sn = stats_pool.tile([128, N], F32)
            nc.vector.tensor_mul(out=sn[:], in0=s_all[:], in1=rec[:])
            wf32 = stats_pool.tile([128, N], F32)
            nc.vector.tensor_mul(out=wf32[:], in0=cs0[:], in1=sn[:])
            wfb = stats_pool.tile([128, N], BF16)
            nc.vector.tensor_copy(out=wfb[:], in_=wf32[:])

        # ---- WREP[y, n, x] = wf[y, n] replicated over x by doubling copies,
        # one n-chunk at a time so the first output tile starts ASAP.
        wrep = wrep_pool.tile([128, N, W], BF16)

        def build_wrep_chunk(c):
            # Only chunk 0 runs at high priority: it gates the first store.
            # Later chunks take their natural program position so the
            # scheduler cannot interleave them into chunk 0's build.
            import contextlib

            sl = slice(c * NC, (c + 1) * NC)
            with hp() if c == 0 else contextlib.nullcontext():
                seed_in = bass.AP(
                    tensor=wfb.tensor,
                    offset=wfb.offset + c * NC,
                    ap=[wfb.ap[0], [1, NC], [1, 1]],
                )
                nc.vector.tensor_copy(out=wrep[:, sl, 0:1], in_=seed_in)
                k = 1
                while k < W:
                    nc.vector.tensor_copy(
                        out=wrep[:, sl, k : 2 * k], in_=wrep[:, sl, 0:k]
                    )
                    k *= 2

        # ---- Phase C: out[u, y, nchunk, x] = mlf[u][y, x] * wrep[y, nchunk, x]
        # (all operands innermost-packed bf16 -> DVE 2x mode; 3 tiles on Pool)
        def emit_tile(u, c):
            sl = slice(c * NC, (c + 1) * NC)
            ot = out_pool.tile(
                [128, NC, W], BF16, name=f"ot{u}_{c}", tag="ot", bufs=OUT_BUFS
            )
            m_b = bass.AP(
                tensor=mlfb[u].tensor,
                offset=mlfb[u].offset,
                ap=[mlfb[u].ap[0], [0, NC], mlfb[u].ap[1]],
            )
            eng = nc.gpsimd if (u, c) in pool_tiles else nc.vector
            eng.tensor_mul(out=ot[:], in0=m_b, in1=wrep[:, sl, :])
            nc.sync.dma_start(out=out_s[u, :, sl, :], in_=ot[:])

        import contextlib

        def chunk_pin(c):
            if WREP_PIN > 0 and c >= 1:
                return tc.tile_wait_until(WREP_PIN + 0.0025 * c)
            return contextlib.nullcontext()

        if WREP_EAGER:
            build_wrep_chunk(0)
            with hp():
                emit_tile(0, 0)
            for c in range(1, N // NC):
                with chunk_pin(c):
                    build_wrep_chunk(c)
        for u in range(U):
            for c in range(N // NC):
                if u == 0 and not WREP_EAGER:
                    with chunk_pin(c):
                        build_wrep_chunk(c)
                if (u, c) == (0, 0):
                    if not WREP_EAGER:
                        with hp():
                            emit_tile(u, c)
                else:
                    emit_tile(u, c)
            if u + 1 < U:
                # pin each remaining V-sum reduce shortly before its mlf is
                # consumed, past the stats->WREP->first-store window, so the
                # scheduler cannot slot the 2.7us bursts into it
                with tc.tile_wait_until(PIN_BASE + PIN_STEP * u):
                    reduce_u(u + 1)


def build_nc():
    nc = bacc.Bacc("TRN2", target_bir_lowering=False, debug=True)
    lfi_s = nc.dram_tensor("lfi_s", [U, HY, W, V], BF16, kind="ExternalInput")
    fm = nc.dram_tensor("fm", [H, W, N], BF16, kind="ExternalInput")
    out_s = nc.dram_tensor("out_s", [U, HY, N, W], BF16, kind="ExternalOutput")
    with tile.TileContext(nc) as tc:
        build_kernel_body(nc, tc, lfi_s, fm, out_s)
    nc.compile()
    return nc


_CACHE = {}


def make_in_maps(lfi, f_maps):
    in_maps = []
    for c in range(8):
        b, half = divmod(c, 2)
        lf = np.ascontiguousarray(
            lfi[b, :, half * HY : (half + 1) * HY]
        ).astype(NP_BF16)
        fmb = f_maps[b]
        mine = fmb[:, half * HY : (half + 1) * HY, :]
        oth = fmb[:, (1 - half) * HY : (2 - half) * HY, :]
        fmc = np.ascontiguousarray(
            np.concatenate([mine, oth], axis=1)
        ).astype(NP_BF16)
        in_maps.append({"lfi_s": lf, "fm": fmc})
    return in_maps


def kernel(lfi, f_maps):
    lfi = np.asarray(lfi, dtype=np.float32)
    f_maps = np.asarray(f_maps, dtype=np.float32)
    if "nc" not in _CACHE:
        _CACHE["nc"] = build_nc()
    nc = _CACHE["nc"]
    res = run_bass_kernel_spmd(nc, make_in_maps(lfi, f_maps), list(range(8)))
    out = np.empty((B, U, H, W, N), np.float32)
    for c in range(8):
        b, half = divmod(c, 2)
        r = np.asarray(res.results[c]["out_s"])  # [U, HY, N, W] bf16
        out[b, :, half * HY : (half + 1) * HY] = r.transpose(0, 1, 3, 2).astype(
            np.float32
        )
    return out


# revision 29
# speedup vs baseline: 1.0009x; 1.0009x over previous
"""DepthCueExtractor TRN2 kernel (bf16 I/O, collective-free).

out[b,u,y,x,n] = sum_v(lfi[b,u,y,x,v]) * wf[b,y,n]
  wf[b,y,n]     = colsum[b,y,n] * s_mask[b,n] / (V * max_w colsum[b,w,n])
  s_mask[b,n]   = sum_{h,w} f_maps[b,h,w,n]
  colsum[b,w,n] = sum_h f_maps[b,h,w,n]   (h==w==256 so w doubles as y)

Sharding: 8 cores = (batch b) x (H-half), data-parallel on the output with no
cross-core communication: each core redundantly reads BOTH 128-column W-halves
of f_maps[b] (+4.2MB that hides under the 37.8MB store stream, cheaper than
the cost model's 15us flat collective overhead), computing the global
per-(b,n) sum/max stats locally.

Everything runs in bf16: inputs are cast on the host (stats still accumulate
in f32 on the PE/PSUM path so only the lfi samples and final product round),
and the output is stored bf16, halving the dominant HBM write stream.
Per-core traffic 51.4MB -> ~143us at the 360GB/s DMA roofline.

colsum is built by ones-stationary PE matmuls accumulating both h-halves into
[1, 2048] PSUM rows (no DVE work; 6 row slots at partition bases 0/32/64 of
two [65, 2048] tiles so rows never wait on each other), evacuated row-by-row
on the otherwise-idle Act engine and scatter-DMA'd into [w-partition, n]
layout via Act's HWDGE queue (so the tiny scatters never head-block the SP
load queue).

The output is laid out [U, Y, N, X] on device (host transposes during the
gather): with x innermost, every operand of the big product - mlf[y,x]
broadcast over an OUTER n dim, and wf[y,n] pre-replicated over x into
WREP[y,n,x] by log-doubling copies - is innermost-packed 2-byte, which
qualifies for the DVE 2x perf mode (0.52ns/elem). One output tile per
u-group goes to the idle Pool engine and the remaining V-sum reduces are
wait_until-pinned near their consumers, keeping the DVE multiply stream
paced to the store stream.
"""

import numpy as np
import ml_dtypes

import concourse.bass as bass
import concourse.bacc as bacc
import concourse.mybir as mybir
import concourse.tile as tile
from concourse.bass_utils import run_bass_kernel_spmd

F32 = mybir.dt.float32
BF16 = mybir.dt.bfloat16
NP_BF16 = ml_dtypes.bfloat16

B, U, H, W, V, N = 4, 9, 256, 256, 9, 64
HY = H // 2
NC = 16  # n-chunk width of one output tile [128, NC, W]
POOL_TILES = None  # resolved in build_kernel_body from NC
EVAC_SPLIT = False   # odd colsum rows evacuated on DVE instead of Act
EVAC_POOL = False    # odd colsum rows evacuated on the idle Pool engine
RED_SPLIT = False    # split V-sum reduces into 4 x-chunks to fit store slack
FM_BUFS = 3
WREP_EAGER = False   # build WREP chunks 1-3 right after first emit
PIN_BASE = 0.030     # ms: wait_until pin for reduce_u(u+1)
PIN_STEP = 0.0115
OUT_BUFS = 6
LT_BUFS = 9
ST_BUFS = 3
WREP_PIN = 0.0      # ms: if >0, pin wrep chunk c>=1 builds at WREP_PIN + 0.0025*c
LT_PIN = 0.0        # ms: if >0, pin lt load u at LT_PIN + 0.00164*u (aligns
                    # scheduler DMA-order estimates with runtime so colsum
                    # matmul sem thresholds don't coalesce onto lfi loads)


def build_kernel_body(nc, tc, lfi_s, fm, out_s):
    import concourse.bass_isa as bass_isa

    pool_tiles = POOL_TILES
    if pool_tiles is None:
        if NC == 16:
            pool_tiles = {(u, 1) for u in range(1, 9)}
        else:
            pool_tiles = {(u, 1) for u in (1, 3, 5, 7)}
    with (
        tc.tile_pool(name="const", bufs=1) as const_pool,
        tc.tile_pool(name="fmp", bufs=1) as fm_pool,
        tc.tile_pool(name="psum", bufs=1, space="PSUM") as psum_pool,
        tc.tile_pool(name="evac", bufs=1) as evac_pool,
        tc.tile_pool(name="stats", bufs=1) as stats_pool,
        tc.tile_pool(name="lfip", bufs=3) as lfi_pool,
        tc.tile_pool(name="mlfp", bufs=1) as mlf_pool,
        tc.tile_pool(name="wrepp", bufs=1) as wrep_pool,
        tc.tile_pool(name="outp", bufs=2) as out_pool,
    ):
        ones = const_pool.tile([128, 1], BF16)
        nc.vector.memset(ones[:], 1.0)

        cs0 = stats_pool.tile([128, N], F32)  # colsum, my w-half (= my y's)
        cs1 = stats_pool.tile([128, N], F32)  # colsum, partner w-half

        # 6 PSUM row slots at partition bases 0/32/64 of two [65, 2048] tiles:
        # every colsum row's matmuls can run as soon as its fm half lands,
        # instead of chaining behind a prior row's evacuation.
        ps_a = psum_pool.tile([65, 32 * N], F32)
        ps_b = psum_pool.tile([65, 32 * N], F32)

        def row_slot(row):
            s = row % 6
            return (ps_a if s < 3 else ps_b), 32 * (s % 3)

        # ---- Phase A: colsum[w, n] = sum_h fm[h, w, n] for all 256 w.
        # Per 64-w chunk: load both h-halves, accumulate them into [1, 2048]
        # PSUM rows with ones-stationary matmuls, copy each row to SBUF on
        # Act, scatter-DMA it to 32 partitions of cs0/cs1.
        def phase_a_chunk(q):
            f0 = fm_pool.tile([128, 64, N], BF16, name=f"f0_{q}", tag="f0", bufs=FM_BUFS)
            f1 = fm_pool.tile([128, 64, N], BF16, name=f"f1_{q}", tag="f1", bufs=FM_BUFS)
            for h in range(2):
                sl = slice(q * 64 + h * 32, q * 64 + (h + 1) * 32)
                tl = slice(h * 32, (h + 1) * 32)
                nc.sync.dma_start(out=f0[:, tl, :], in_=fm[0:128, sl, :])
                nc.sync.dma_start(out=f1[:, tl, :], in_=fm[128:256, sl, :])
            for r2 in range(2):
                row = 2 * q + r2
                pt, pb = row_slot(row)
                rt = pt[pb : pb + 1, :]
                for sg in range(4):
                    osl = slice(sg * 512, (sg + 1) * 512)
                    rsl = slice(r2 * 32 + sg * 8, r2 * 32 + (sg + 1) * 8)
                    nc.tensor.matmul(
                        out=rt[0:1, osl], lhsT=ones[:, 0:1],
                        rhs=f0[:, rsl, :], start=True, stop=False,
                    )
                    nc.tensor.matmul(
                        out=rt[0:1, osl], lhsT=ones[:, 0:1],
                        rhs=f1[:, rsl, :], start=False, stop=True,
                    )
                st = evac_pool.tile(
                    [1, 32 * N], F32, name=f"st{row}", tag="st", bufs=ST_BUFS
                )
                # alternate evac rows between Act and (otherwise idle) DVE so
                # the eight 1.9us copies don't serialize on one engine
                if row % 2 == 1 and EVAC_POOL:
                    nc.gpsimd.tensor_copy(out=st[:], in_=rt[0:1, :])
                elif row % 2 == 1 and EVAC_SPLIT:
                    nc.vector.tensor_copy(out=st[:], in_=rt[0:1, :])
                else:
                    nc.scalar.copy(out=st[:], in_=rt[0:1, :])
                dst = cs0 if row < 4 else cs1
                pbase = (row % 4) * 32
                # Act's HWDGE queue: keeps these off the SP queue so they
                # never head-block the input load stream.
                nc.scalar.dma_start(out=dst[pbase : pbase + 32, :], in_=st[0:1, :])

        # All fm chunks load FIRST: the colsum/stats path (which gates the
        # first store through wf->WREP) completes while the lfi loads are
        # still streaming, so the store stream starts before inputs finish.
        for q in range(4):
            phase_a_chunk(q)

        lts = []

        def load_lt(u):
            # bufs=9: every lfi tile stays live; reduces are pinned late, so
            # recycling would stall the load stream.
            lt = lfi_pool.tile([128, W, V], BF16, name=f"lt{u}", tag="lt", bufs=LT_BUFS)
            nc.sync.dma_start(out=lt[:], in_=lfi_s[u])
            lts.append(lt)

        for u in range(U):
            if LT_PIN > 0:
                with tc.tile_wait_until(LT_PIN + 0.00164 * u):
                    load_lt(u)
            else:
                load_lt(u)

        mlf32 = [
            mlf_pool.tile([128, W], F32, name=f"m32_{u}", tag="m32", bufs=2)
            for u in range(U)
        ]
        mlfb = [
            mlf_pool.tile([128, W], BF16, name=f"mb{u}", tag=f"mb{u}")
            for u in range(U)
        ]

        def reduce_u(u):
            if RED_SPLIT:
                for xc in range(4):
                    xs = slice(xc * 64, (xc + 1) * 64)
                    nc.vector.reduce_sum(
                        out=mlf32[u][:, xs], in_=lts[u][:, xs, :],
                        axis=mybir.AxisListType.X,
                    )
                    nc.vector.tensor_copy(
                        out=mlfb[u][:, xs], in_=mlf32[u][:, xs]
                    )
            else:
                nc.vector.reduce_sum(
                    out=mlf32[u][:], in_=lts[u][:], axis=mybir.AxisListType.X
                )
                nc.vector.tensor_copy(out=mlfb[u][:], in_=mlf32[u][:])

        reduce_u(0)  # on the first-store critical path; the rest are pinned

        hp = tc.high_priority

        # ---- stats: global sum/max over w, then wf = cs_my * s / (V * max)
        with hp():
            t_sum = stats_pool.tile([128, N], F32)
            nc.vector.tensor_add(out=t_sum[:], in0=cs0[:], in1=cs1[:])
            t_max = stats_pool.tile([128, N], F32)
            nc.vector.tensor_max(out=t_max[:], in0=cs0[:], in1=cs1[:])
            s_all = stats_pool.tile([128, N], F32)
            nc.gpsimd.partition_all_reduce(
                s_all[:], t_sum[:], 128, bass_isa.ReduceOp.add
            )
            m_all = stats_pool.tile([128, N], F32)
            nc.gpsimd.partition_all_reduce(
                m_all[:], t_max[:], 128, bass_isa.ReduceOp.max
            )
            m9 = stats_pool.tile([128, N], F32)
            nc.vector.tensor_scalar_mul(m9[:], m_all[:], float(V))
            rec = stats_pool.tile([128, N], F32)
            nc.vector.reciprocal(out=rec[:], in_=m9[:])
            sn = stats_pool.tile([128, N], F32)
            nc.vector.tensor_mul(out=sn[:], in0=s_all[:], in1=rec[:])
            wf32 = stats_pool.tile([128, N], F32)
            nc.vector.tensor_mul(out=wf32[:], in0=cs0[:], in1=sn[:])
            wfb = stats_pool.tile([128, N], BF16)
            nc.vector.tensor_copy(out=wfb[:], in_=wf32[:])

        # ---- WREP[y, n, x] = wf[y, n] replicated over x by doubling copies,
        # one n-chunk at a time so the first output tile starts ASAP.
        wrep = wrep_pool.tile([128, N, W], BF16)

        def build_wrep_chunk(c):
            # Only chunk 0 runs at high priority: it gates the first store.
            # Later chunks take their natural program position so the
            # scheduler cannot interleave them into chunk 0's build.
            import contextlib

            sl = slice(c * NC, (c + 1) * NC)
            with hp() if c == 0 else contextlib.nullcontext():
                seed_in = bass.AP(
                    tensor=wfb.tensor,
                    offset=wfb.offset + c * NC,
                    ap=[wfb.ap[0], [1, NC], [1, 1]],
                )
                nc.vector.tensor_copy(out=wrep[:, sl, 0:1], in_=seed_in)
                k = 1
                while k < W:
                    nc.vector.tensor_copy(
                        out=wrep[:, sl, k : 2 * k], in_=wrep[:, sl, 0:k]
                    )
                    k *= 2

        # ---- Phase C: out[u, y, nchunk, x] = mlf[u][y, x] * wrep[y, nchunk, x]
        # (all operands innermost-packed bf16 -> DVE 2x mode; 3 tiles on Pool)
        def emit_tile(u, c):
            sl = slice(c * NC, (c + 1) * NC)
            ot = out_pool.tile(
                [128, NC, W], BF16, name=f"ot{u}_{c}", tag="ot", bufs=OUT_BUFS
            )
            m_b = bass.AP(
                tensor=mlfb[u].tensor,
                offset=mlfb[u].offset,
                ap=[mlfb[u].ap[0], [0, NC], mlfb[u].ap[1]],
            )
            eng = nc.gpsimd if (u, c) in pool_tiles else nc.vector
            eng.tensor_mul(out=ot[:], in0=m_b, in1=wrep[:, sl, :])
            nc.sync.dma_start(out=out_s[u, :, sl, :], in_=ot[:])

        import contextlib

        def chunk_pin(c):
            if WREP_PIN > 0 and c >= 1:
                return tc.tile_wait_until(WREP_PIN + 0.0025 * c)
            return contextlib.nullcontext()

        if WREP_EAGER:
            build_wrep_chunk(0)
            with hp():
                emit_tile(0, 0)
            for c in range(1, N // NC):
                with chunk_pin(c):
                    build_wrep_chunk(c)
        for u in range(U):
            for c in range(N // NC):
                if u == 0 and not WREP_EAGER:
                    with chunk_pin(c):
                        build_wrep_chunk(c)
                if (u, c) == (0, 0):
                    if not WREP_EAGER:
                        with hp():
                            emit_tile(u, c)
                else:
                    emit_tile(u, c)
            if u + 1 < U:
                # pin each remaining V-sum reduce shortly before its mlf is
                # consumed, past the stats->WREP->first-store window, so the
                # scheduler cannot slot the 2.7us bursts into it
                with tc.tile_wait_until(PIN_BASE + PIN_STEP * u):
                    reduce_u(u + 1)


def build_nc():
    nc = bacc.Bacc("TRN2", target_bir_lowering=False, debug=True)
    lfi_s = nc.dram_tensor("lfi_s", [U, HY, W, V], BF16, kind="ExternalInput")
    fm = nc.dram_tensor("fm", [H, W, N], BF16, kind="ExternalInput")
    out_s = nc.dram_tensor("out_s", [U, HY, N, W], BF16, kind="ExternalOutput")
    with tile.TileContext(nc) as tc:
        build_kernel_body(nc, tc, lfi_s, fm, out_s)
    nc.compile()
    return nc


_CACHE = {}


def make_in_maps(lfi, f_maps):
    in_maps = []
    for c in range(8):
        b, half = divmod(c, 2)
        lf = np.ascontiguousarray(
            lfi[b, :, half * HY : (half + 1) * HY]
        ).astype(NP_BF16)
        fmb = f_maps[b]
        mine = fmb[:, half * HY : (half + 1) * HY, :]
        oth = fmb[:, (1 - half) * HY : (2 - half) * HY, :]
        fmc = np.ascontiguousarray(
            np.concatenate([mine, oth], axis=1)
        ).astype(NP_BF16)
        in_maps.append({"lfi_s": lf, "fm": fmc})
    return in_maps


def kernel(lfi, f_maps):
    lfi = np.asarray(lfi, dtype=np.float32)
    f_maps = np.asarray(f_maps, dtype=np.float32)
    if "nc" not in _CACHE:
        _CACHE["nc"] = build_nc()
    nc = _CACHE["nc"]
    res = run_bass_kernel_spmd(nc, make_in_maps(lfi, f_maps), list(range(8)))
    out = np.empty((B, U, H, W, N), np.float32)
    for c in range(8):
        b, half = divmod(c, 2)
        r = np.asarray(res.results[c]["out_s"])  # [U, HY, N, W] bf16
        out[b, :, half * HY : (half + 1) * HY] = r.transpose(0, 1, 3, 2).astype(
            np.float32
        )
    return out
